# revision 31
# baseline (speedup 1.0000x reference)
"""Bass/Tile TRN2 kernel for nn_BertSelfAttention2 (B=2, S=2048, D=1024, H=16).

Sharding: 8 cores = 2 (batch) x 4 (head groups of 4 heads). Each core
computes Q/K projections for its 4 heads (as 2 packed pairs), the modified
attention (kt = softplus(k), v = q + k, mask on the query axis), and writes
its slice of the output.

Everything is computed in "T" orientation (scoresT[k, q]) so no large
on-device transposes are needed; the final [head*64, S] -> [S, 256]
transpose happens on the host in _gather.

The query-axis mask is exploited rather than applied: every masked query
produces the SAME output (uniform softmax = mean(v)), so only the unmasked
query columns are gathered (gpsimd ap_gather) into a packed set, plus one
reserved zeroed column whose result the host broadcasts to all masked
positions.  With ~50% masked queries this cuts scores/exp/ctx work by 25%
(4x512 -> 4x384 packed columns).

All matmul operands are bf16 (fp32r streams ~2.3 cycles/col on TRN2's PE,
bf16 streams 1); accumulation stays fp32 in PSUM.  kt = ln(exp(k)+1) runs
entirely under one activation table (natural_log_exp_and_others, loaded
explicitly once) so the scalar engine never thrashes table reloads.  The
projections of group g+1 are interleaved into attention of group g so the
PE and ACT engines both stay fed (also keeps the PE activity monitor from
down-clocking 2.4 -> 1.2 GHz).  Inputs are shipped pre-tiled and packed so
the whole input load is 8 large contiguous DMAs.
"""
import sys

if "/opt/trn_rl_repo" not in sys.path:
    sys.path.insert(0, "/opt/trn_rl_repo")

import numpy as np
import ml_dtypes

B, S, D = 2, 2048, 1024
H = 16
HD = 64
NCORES = 8
HPC = H // (NCORES // B)     # heads per core = 4
NG = HPC // 2                # head-pair groups per core = 2
SC = 4                       # 512-wide seq chunks
KC = S // 128                # 16 key chunks

_CACHE = {}
_META = {}


def _build(cap, zero_col0):
    """cap: packed query columns kept per 512-seq chunk (%16==0; 384
    normally).  zero_col0: zero packed column 0 (the masked-query rep)."""
    import concourse.tile as tile
    from concourse import bacc, mybir
    from concourse.hw_specs import get_activation_tables

    F32 = mybir.dt.float32
    BF16 = mybir.dt.bfloat16
    I16 = mybir.dt.int16
    AF = mybir.ActivationFunctionType

    NQ = cap * SC                # packed query columns (1536 normally)
    SQC = NQ // 512              # packed 512-col matmul chunks (3 normally)
    assert NQ % 512 == 0 and cap % 16 == 0

    nc = bacc.Bacc(None, target_bir_lowering=False, debug=False)

    # pre-tiled packed operands: each load is one big contiguous DMA
    xt = nc.declare_dram_parameter("xt", [SC * 128, 8 * 512], BF16, isOutput=False)
    wq = nc.declare_dram_parameter("wq", [NG * 128, 8 * 128], BF16, isOutput=False)
    wk = nc.declare_dram_parameter("wk", [NG * 128, 8 * 128], BF16, isOutput=False)
    bq = nc.declare_dram_parameter("bq", [2 * 128], F32, isOutput=False)
    bk = nc.declare_dram_parameter("bk", [2 * 128], F32, isOutput=False)
    qidx = nc.declare_dram_parameter("qidx", [128, SC * (cap // 16)], I16,
                                     isOutput=False)
    out = nc.declare_dram_parameter("out", [HPC * HD, NQ], F32, isOutput=True)

    with tile.TileContext(nc) as tc:
        with tc.tile_pool(name="consts", bufs=1) as consts, \
             tc.tile_pool(name="big", bufs=1) as big, \
             tc.tile_pool(name="tmp", bufs=2) as tmp, \
             tc.tile_pool(name="expp", bufs=3) as expp, \
             tc.tile_pool(name="ep", bufs=2) as ep, \
             tc.tile_pool(name="ps_s", bufs=2, space="PSUM") as ps_s, \
             tc.tile_pool(name="ps_c", bufs=1, space="PSUM") as ps_c, \
             tc.tile_pool(name="ps_m", bufs=2, space="PSUM") as ps_m:

            # load the Exp+Ln activation table once, explicitly: with both
            # functions resident the act-table pass inserts no reloads
            if True:  # BISECT: manual combined-table load
                tabs = list(get_activation_tables(nc.m.arch))
                nl_exp_id = tabs.index("natural_log_exp_and_others")
                nc.scalar.add_instruction(mybir.InstLoadActFuncSet(
                    name=nc.get_next_instruction_name(), ins=[], outs=[],
                    act_func_set_id=nl_exp_id))

            # ---- input loads: 8 large DMAs on sync, 5 tiny on gpsimd ----
            wq_t = [consts.tile([128, 1024], BF16, tag=f"wq{g}", name=f"wq{g}")
                    for g in range(NG)]
            wk_t = [consts.tile([128, 1024], BF16, tag=f"wk{g}", name=f"wk{g}")
                    for g in range(NG)]
            xt_t = [big.tile([128, 4096], BF16, tag=f"xt{scc}", name=f"xt{scc}")
                    for scc in range(SC)]
            qidx_t = consts.tile([128, SC * (cap // 16)], I16, tag="qi",
                                 name="qi")

            # all bulk input goes on the sync (SP) hardware-dynamic queue:
            # it fans out across 6+ DMA engines at ~150 GB/s, unlike the
            # scalar/gpsimd queues which drain through a single engine
            nc.sync.dma_start(out=wq_t[0], in_=wq[0:128, :])
            nc.sync.dma_start(out=xt_t[0], in_=xt[0:128, :])
            nc.sync.dma_start(out=wk_t[0], in_=wk[0:128, :])
            nc.sync.dma_start(out=xt_t[1], in_=xt[128:256, :])
            nc.sync.dma_start(out=wq_t[1], in_=wq[128:256, :])
            nc.sync.dma_start(out=wk_t[1], in_=wk[128:256, :])
            nc.sync.dma_start(out=xt_t[2], in_=xt[256:384, :])
            nc.sync.dma_start(out=xt_t[3], in_=xt[384:512, :])

            bq_t, bk_t = [], []
            for g in range(NG):
                bqt = consts.tile([128, 1], F32, tag=f"bq{g}", name=f"bq{g}")
                nc.gpsimd.dma_start(
                    out=bqt,
                    in_=bq[g * 128:(g + 1) * 128].rearrange("(p o) -> p o", o=1))
                bq_t.append(bqt)
                bkt = consts.tile([128, 1], F32, tag=f"bk{g}", name=f"bk{g}")
                nc.gpsimd.dma_start(
                    out=bkt,
                    in_=bk[g * 128:(g + 1) * 128].rearrange("(p o) -> p o", o=1))
                bk_t.append(bkt)
            nc.gpsimd.dma_start(out=qidx_t, in_=qidx[:, :])

            # identity for the small V transposes
            from concourse.masks import make_identity
            ident = consts.tile([128, 128], F32)
            make_identity(nc, ident)

            # ---- persistent activations ----
            tqf = [big.tile([128, S], F32, tag=f"tqf{g}", name=f"tqf{g}")
                   for g in range(NG)]
            qg = [big.tile([128, NQ], F32, tag=f"qg{g}", name=f"qg{g}")
                  for g in range(NG)]
            # qtp: packed q per head, bf16; the other head's rows stay zero
            # so the scores matmuls run a full K=128 contraction
            qtp = [[big.tile([128, NQ], BF16, tag=f"qtp{g}_{hh}",
                             name=f"qtp{g}_{hh}") for hh in range(2)]
                   for g in range(NG)]
            for g in range(NG):
                nc.vector.memset(qtp[g][0][64:128, :], 0.0)
                nc.vector.memset(qtp[g][1][0:64, :], 0.0)
            kt = [[big.tile([128, 512], BF16, tag=f"kt{g}_{sc}",
                            name=f"kt{g}_{sc}") for sc in range(SC)]
                  for g in range(NG)]
            # vp: v^T per head per key chunk + a ones column (denominator)
            vp = [[big.tile([128, 65], BF16, tag=f"vp{h}_{kc}",
                            name=f"vp{h}_{kc}") for kc in range(KC)]
                  for h in range(HPC)]

            def proj_chunks(g):
                """Generator: emits group g's projection; yields after every
                PE instruction so attention of group g-1 can interleave."""
                for sc in range(SC):
                    ssl = slice(sc * 512, (sc + 1) * 512)
                    csl = slice(sc * cap, (sc + 1) * cap)
                    isl = slice(sc * (cap // 16), (sc + 1) * (cap // 16))
                    pq = ps_m.tile([128, 512], F32, tag="ep", name=f"pq{g}_{sc}")
                    for dc in range(8):
                        nc.tensor.matmul(pq[:, 0:512],
                                         wq_t[g][:, dc * 128:(dc + 1) * 128],
                                         xt_t[sc][:, dc * 512:(dc + 1) * 512],
                                         start=(dc == 0), stop=(dc == 7))
                        yield
                    pk = ps_m.tile([128, 512], F32, tag="ep", name=f"pk{g}_{sc}")
                    for dc in range(8):
                        nc.tensor.matmul(pk[:, 0:512],
                                         wk_t[g][:, dc * 128:(dc + 1) * 128],
                                         xt_t[sc][:, dc * 512:(dc + 1) * 512],
                                         start=(dc == 0), stop=(dc == 7))
                        yield
                    nc.vector.tensor_scalar_add(tqf[g][:, ssl], pq[:, 0:512],
                                                bq_t[g])
                    tk = tmp.tile([128, 512], F32, tag="tk", name=f"tk{g}_{sc}")
                    nc.vector.tensor_scalar_add(tk, pk[:, 0:512], bk_t[g])
                    # kt = softplus(k) = ln(exp(k) + 1), one table resident
                    te = tmp.tile([128, 512], F32, tag="te", name=f"te{g}_{sc}")
                    nc.scalar.activation(out=te, in_=tk, func=AF.Exp)
                    nc.scalar.activation(out=kt[g][sc], in_=te,
                                         func=AF.Ln, bias=1.0)
                    # v = q + k
                    vts = tmp.tile([128, 512], F32, tag="vts",
                                   name=f"vts{g}_{sc}")
                    nc.vector.tensor_add(vts, tqf[g][:, ssl], tk)
                    # packed-q gather for this chunk (gpsimd); the bf16
                    # casts into qtp are deferred until every gather is done
                    # so no DVE instruction ever waits on the gpsimd queue
                    # (a waiting DVE op stalls the DVE completion count that
                    # the PE transposes and scalar exps wait on)
                    nc.gpsimd.ap_gather(
                        qg[g][:, csl], tqf[g][:, ssl], qidx_t[:, isl],
                        channels=128, num_elems=512, d=1, num_idxs=cap)
                    # v^T: one [128,128] PE transpose per key chunk serves
                    # both heads
                    for jj in range(4):
                        j = sc * 4 + jj
                        pv = ps_m.tile([128, 128], F32, tag="ep",
                                       name=f"pv{g}_{j}")
                        nc.tensor.transpose(pv, vts[:, jj * 128:(jj + 1) * 128],
                                            ident)
                        yield
                        for hh in range(2):
                            h = g * 2 + hh
                            nc.vector.tensor_copy(vp[h][j][:, 0:64],
                                                  pv[:, hh * 64:(hh + 1) * 64])
                            nc.vector.memset(vp[h][j][:, 64:65], 1.0)
                for sc in range(SC):
                    csl = slice(sc * cap, (sc + 1) * cap)
                    nc.vector.tensor_copy(qtp[g][0][0:64, csl],
                                          qg[g][0:64, csl])
                    nc.vector.tensor_copy(qtp[g][1][64:128, csl],
                                          qg[g][64:128, csl])
                if zero_col0:
                    nc.vector.memset(qtp[g][0][0:64, 0:1], 0.0)
                    nc.vector.memset(qtp[g][1][64:128, 0:1], 0.0)

            def drain(gen):
                if gen is not None:
                    next(gen, None)

            def make_epilogue(g, qc, csA, csB):
                # everything here reads the SBUF copies, so it can run long
                # after the PSUM accumulators were reused
                def emit(chunks):
                    # 1/den = exp(-ln(den)) on the scalar engine (resident
                    # table, slots into the exp stream); broadcast AND the
                    # final multiply run on gpsimd so no DVE instruction
                    # waits cross-engine.  Broadcasts then multiplies are
                    # batched to limit gpsimd library reloads.
                    for c0, c1 in chunks:
                        w = c1 - c0
                        bcs = []
                        for hh, cs in ((0, csA), (1, csB)):
                            lnd = ep.tile([1, 512], F32, tag=f"l{hh}",
                                          name=f"l{g}_{qc}_{hh}_{c0}")
                            nc.scalar.activation(out=lnd[:, 0:w],
                                                 in_=cs[64:65, c0:c1],
                                                 func=AF.Ln)
                            rcp = ep.tile([1, 512], F32, tag=f"r{hh}",
                                          name=f"r{g}_{qc}_{hh}_{c0}")
                            nc.scalar.activation(out=rcp[:, 0:w],
                                                 in_=lnd[:, 0:w],
                                                 func=AF.Exp, scale=-1.0)
                            bc = ep.tile([64, 512], F32, tag=f"b{hh}",
                                         name=f"b{g}_{qc}_{hh}_{c0}")
                            nc.gpsimd.partition_broadcast(bc[:, 0:w],
                                                          rcp[0:1, 0:w])
                            bcs.append(bc)
                        for hh, cs in ((0, csA), (1, csB)):
                            cf = ep.tile([64, 512], F32, tag=f"cf{hh}",
                                         name=f"cf{g}_{qc}_{hh}_{c0}")
                            nc.gpsimd.tensor_tensor(
                                out=cf[:, 0:w], in0=cs[0:64, c0:c1],
                                in1=bcs[hh][:, 0:w],
                                op=mybir.AluOpType.mult)
                            row = (g * 2 + hh) * 64
                            nc.gpsimd.dma_start(
                                out=out[row:row + 64,
                                        qc * 512 + c0:qc * 512 + c1],
                                in_=cf[:, 0:w])
                return emit

            def attn_group(g, gen, pending):
                """Attention for group g; drains proj of group g+1 between
                matmuls to keep the PE fed while the ACT engine exps."""
                vpA = vp[g * 2]
                vpB = vp[g * 2 + 1]
                ktg = kt[g]
                for qc in range(SQC):
                    qsl = slice(qc * 512, (qc + 1) * 512)
                    cA = ps_c.tile([65, 512], F32, tag="cA", name=f"cA{g}_{qc}")
                    cB = ps_c.tile([65, 512], F32, tag="cB", name=f"cB{g}_{qc}")
                    prev = None
                    for kc in range(KC):
                        sAB = ps_s.tile([128, 1024], F32, tag="sAB",
                                        name=f"s{g}_{qc}_{kc}")
                        lhs = ktg[kc // 4][:, (kc % 4) * 128:(kc % 4 + 1) * 128]
                        nc.tensor.matmul(sAB[:, 0:512], lhs,
                                         qtp[g][0][:, qsl],
                                         start=True, stop=True)
                        nc.tensor.matmul(sAB[:, 512:1024], lhs,
                                         qtp[g][1][:, qsl],
                                         start=True, stop=True)
                        eAB = expp.tile([128, 1024], BF16, tag="eAB",
                                        name=f"e{g}_{qc}_{kc}")
                        nc.scalar.activation(out=eAB, in_=sAB, func=AF.Exp,
                                             scale=0.125)
                        drain(gen)
                        if kc == 8 and pending[0] is not None:
                            # previous qc's deferred normalize: by now the
                            # interleaved proj's DVE work has cleared, so the
                            # slow reciprocal doesn't head-of-line block it
                            pending[0]([(0, 512)])
                            pending[0] = None
                        # ctx runs one kc behind so the PE never waits on exp
                        if prev is not None:
                            pe, pkc = prev
                            nc.tensor.matmul(cA, vpA[pkc], pe[:, 0:512],
                                             start=(pkc == 0), stop=False)
                            nc.tensor.matmul(cB, vpB[pkc], pe[:, 512:1024],
                                             start=(pkc == 0), stop=False)
                        drain(gen)
                        prev = (eAB, kc)
                    pe, pkc = prev
                    nc.tensor.matmul(cA, vpA[pkc], pe[:, 0:512],
                                     start=False, stop=True)
                    nc.tensor.matmul(cB, vpB[pkc], pe[:, 512:1024],
                                     start=False, stop=True)
                    # copy ctx+denominator to SBUF immediately: this frees
                    # the PSUM accumulator so the next qc never stalls; the
                    # slow normalize is deferred into the next qc
                    csA = ep.tile([65, 512], F32, tag="cs0",
                                  name=f"cs{g}_{qc}_0")
                    nc.vector.tensor_copy(csA, cA)
                    csB = ep.tile([65, 512], F32, tag="cs1",
                                  name=f"cs{g}_{qc}_1")
                    nc.vector.tensor_copy(csB, cB)
                    pending[0] = make_epilogue(g, qc, csA, csB)

            gen0 = proj_chunks(0)
            for _ in gen0:
                pass
            pending = [None]
            for g in range(NG):
                gen_next = proj_chunks(g + 1) if g + 1 < NG else None
                attn_group(g, gen_next, pending)
                if gen_next is not None:
                    for _ in gen_next:
                        pass
            # tail: the last qc's normalize, chunked so reciprocal/broadcast/
            # multiply/store pipeline instead of serializing ~7us
            pending[0]([(0, 128), (128, 256), (256, 384), (384, 512)])

    nc.finalize()
    return nc


def _get_nc(cap, zero_col0):
    key = (cap, zero_col0)
    if key not in _CACHE:
        _CACHE[key] = _build(cap, zero_col0)
    return _CACHE[key]


def _pack_queries(am_row, cap):
    """Build per-512-chunk packed index lists for one batch.

    Packed column c*cap+s <- query (c*512 + idx[c][s]).  Column 0 is
    always reserved and zeroed on the device; every masked query position
    takes its output from column 0 on the host (a zeroed q column yields
    the uniform-softmax result, identical for all masked queries).

    Returns (fits, wrapped_idx [128, SC*cap//16] int16, scatter info).
    """
    masked = np.where(am_row == 0)[0]
    cols = []          # packed column (valid entries, in order)
    qpos = []          # matching global query index
    wrapped = np.zeros((SC, 128, cap // 16), dtype=np.int16)
    for c in range(SC):
        lo = c * 512
        un = np.where(am_row[lo:lo + 512] == 1)[0]    # local indices
        reserve = 1 if c == 0 else 0
        if len(un) + reserve > cap:
            return False, None, None
        idx = np.zeros(cap, dtype=np.int16)           # pad/rep = 0 (valid)
        idx[reserve:reserve + len(un)] = un
        # wrapped layout: index j lives at [j % 16, j // 16], replicated
        # into each 16-partition block
        wrapped[c] = np.tile(idx.reshape(cap // 16, 16).T, (8, 1))
        cols.extend(c * cap + reserve + i for i in range(len(un)))
        qpos.extend(lo + int(u) for u in un)
    info = {
        "cols": np.asarray(cols, dtype=np.int64),
        "qpos": np.asarray(qpos, dtype=np.int64),
        "masked": masked,
    }
    return True, wrapped.transpose(1, 0, 2).reshape(128, SC * (cap // 16)), info


def _shard_inputs(hidden_states, attention_mask, Wq, bq, Wk, bk):
    hs = np.asarray(hidden_states, dtype=np.float32)
    am = np.asarray(attention_mask)
    Wq = np.asarray(Wq, dtype=np.float32)
    Wk = np.asarray(Wk, dtype=np.float32)
    bq = np.asarray(bq, dtype=np.float32)
    bk = np.asarray(bk, dtype=np.float32)
    BF = ml_dtypes.bfloat16

    # packed capacity: 384/chunk for the random ~50% mask; escalate if a
    # chunk has too many unmasked queries (640 always fits: 512+1 <= 640)
    packs = None
    for cap in (384, 512, 640):
        packs = []
        for b in range(B):
            ok, wrapped, info = _pack_queries(am[b], cap)
            if not ok:
                packs = None
                break
            packs.append((wrapped, info))
        if packs is not None:
            break
    assert packs is not None

    _META.clear()
    _META["cap"] = cap
    _META["zero_col0"] = True
    _META["packs"] = packs

    # X^T packed per seq chunk: [sc][p, dc*512+s] = X[b, sc*512+s, dc*128+p]
    xts = [np.ascontiguousarray(
        hs[b].T.astype(BF).reshape(8, 128, SC, 512).transpose(2, 1, 0, 3)
        .reshape(SC * 128, 8 * 512)) for b in range(B)]

    in_maps = []
    for c in range(NCORES):
        b = c // (NCORES // B)
        hg = c % (NCORES // B)
        cols = slice(hg * 2 * 128, (hg + 1) * 2 * 128)

        def _tile_w(W):
            # [g][p, dc*128+j] = W[dc*128+p, cols[g*128+j]]
            return np.ascontiguousarray(
                W[:, cols].astype(BF).reshape(8, 128, NG, 128)
                .transpose(2, 1, 0, 3).reshape(NG * 128, 8 * 128))
        in_maps.append({
            "xt": xts[b],
            "wq": _tile_w(Wq),
            "wk": _tile_w(Wk),
            "bq": np.ascontiguousarray(bq[cols]),
            "bk": np.ascontiguousarray(bk[cols]),
            "qidx": packs[b][0],
        })
    return in_maps


def _gather(results):
    cap = _META["cap"]
    NQ = cap * SC
    full = np.empty((B, S, D), dtype=np.float32)
    for c in range(NCORES):
        b = c // (NCORES // B)
        hg = c % (NCORES // B)
        _, info = _META["packs"][b]
        r = results[c]["out"].reshape(HPC, HD, NQ)
        for h in range(HPC):
            col = hg * 2 * 128 + h * 64
            blk = full[b, :, col:col + 64]
            blk[info["qpos"], :] = r[h][:, info["cols"]].T
            if len(info["masked"]):
                blk[info["masked"], :] = r[h][:, 0]
    return full


def run_sharded(in_maps, **kw):
    from concourse.bass_utils import run_bass_kernel_spmd
    nc = _get_nc(_META["cap"], _META["zero_col0"])
    return run_bass_kernel_spmd(nc, in_maps, list(range(NCORES)), **kw)


def kernel(hidden_states, attention_mask, Wq, bq, Wk, bk):
    in_maps = _shard_inputs(hidden_states, attention_mask, Wq, bq, Wk, bk)
    res = run_sharded(in_maps)
    return _gather(res.results)


# revision 33
# speedup vs baseline: 1.5938x; 1.5938x over previous
"""Bass/Tile TRN2 kernel for nn_BertSelfAttention2 (B=2, S=2048, D=1024, H=16).

Sharding: 8 cores = 2 (batch) x 4 (head groups of 4 heads). Each core
computes Q/K projections for its 4 heads (as 2 packed pairs), the modified
attention (kt = softplus(k), v = q + k, mask on the query axis), and writes
its slice of the output.

Everything is computed in "T" orientation (scoresT[k, q]) so no large
on-device transposes are needed; the final [head*64, S] -> [S, 256]
transpose happens on the host in _gather.

The query-axis mask is exploited rather than applied: every masked query
produces the SAME output (uniform softmax = mean(v)), so only the unmasked
query columns are gathered (gpsimd ap_gather) into a packed set, plus one
reserved zeroed column whose result the host broadcasts to all masked
positions.  With ~50% masked queries this cuts scores/exp/ctx work by 25%
(4x512 -> 4x384 packed columns).

All matmul operands are bf16 (fp32r streams ~2.3 cycles/col on TRN2's PE,
bf16 streams 1); accumulation stays fp32 in PSUM.  kt = ln(exp(k)+1) runs
entirely under one activation table (natural_log_exp_and_others, loaded
explicitly once) so the scalar engine never thrashes table reloads.  The
projections of group g+1 are interleaved into attention of group g so the
PE and ACT engines both stay fed (also keeps the PE activity monitor from
down-clocking 2.4 -> 1.2 GHz).  Inputs are shipped pre-tiled and packed so
the whole input load is 8 large contiguous DMAs.
"""
import sys

if "/opt/trn_rl_repo" not in sys.path:
    sys.path.insert(0, "/opt/trn_rl_repo")

import numpy as np
import ml_dtypes

B, S, D = 2, 2048, 1024
H = 16
HD = 64
NCORES = 8
HPC = H // (NCORES // B)     # heads per core = 4
NG = HPC // 2                # head-pair groups per core = 2
SC = 4                       # 512-wide seq chunks
KC = S // 128                # 16 key chunks

_CACHE = {}
_META = {}


def _build(cap, zero_col0):
    """cap: packed query columns kept per 512-seq chunk (%16==0; 384
    normally).  zero_col0: zero packed column 0 (the masked-query rep)."""
    import concourse.tile as tile
    from concourse import bacc, mybir
    from concourse.hw_specs import get_activation_tables

    F32 = mybir.dt.float32
    BF16 = mybir.dt.bfloat16
    I16 = mybir.dt.int16
    AF = mybir.ActivationFunctionType

    NQ = cap * SC                # packed query columns (1536 normally)
    SQC = NQ // 512              # packed 512-col matmul chunks (3 normally)
    assert NQ % 512 == 0 and cap % 16 == 0

    nc = bacc.Bacc(None, target_bir_lowering=False, debug=False)

    # pre-tiled packed operands: each load is one big contiguous DMA
    xt = nc.declare_dram_parameter("xt", [SC * 128, 8 * 512], BF16, isOutput=False)
    wq = nc.declare_dram_parameter("wq", [NG * 128, 8 * 128], BF16, isOutput=False)
    wk = nc.declare_dram_parameter("wk", [NG * 128, 8 * 128], BF16, isOutput=False)
    bq = nc.declare_dram_parameter("bq", [2 * 128], F32, isOutput=False)
    bk = nc.declare_dram_parameter("bk", [2 * 128], F32, isOutput=False)
    qidx = nc.declare_dram_parameter("qidx", [128, SC * (cap // 16)], I16,
                                     isOutput=False)
    out = nc.declare_dram_parameter("out", [HPC * HD, NQ], F32, isOutput=True)

    with tile.TileContext(nc) as tc:
        with tc.tile_pool(name="consts", bufs=1) as consts, \
             tc.tile_pool(name="big", bufs=1) as big, \
             tc.tile_pool(name="tmp", bufs=2) as tmp, \
             tc.tile_pool(name="expp", bufs=3) as expp, \
             tc.tile_pool(name="ep", bufs=2) as ep, \
             tc.tile_pool(name="ps_s", bufs=2, space="PSUM") as ps_s, \
             tc.tile_pool(name="ps_c", bufs=1, space="PSUM") as ps_c, \
             tc.tile_pool(name="ps_m", bufs=2, space="PSUM") as ps_m:

            # load the Exp+Ln activation table once, explicitly: with both
            # functions resident the act-table pass inserts no reloads
            if True:  # BISECT: manual combined-table load
                tabs = list(get_activation_tables(nc.m.arch))
                nl_exp_id = tabs.index("natural_log_exp_and_others")
                nc.scalar.add_instruction(mybir.InstLoadActFuncSet(
                    name=nc.get_next_instruction_name(), ins=[], outs=[],
                    act_func_set_id=nl_exp_id))

            # ---- input loads: 8 large DMAs on sync, 5 tiny on gpsimd ----
            wq_t = [consts.tile([128, 1024], BF16, tag=f"wq{g}", name=f"wq{g}")
                    for g in range(NG)]
            wk_t = [consts.tile([128, 1024], BF16, tag=f"wk{g}", name=f"wk{g}")
                    for g in range(NG)]
            xt_t = [big.tile([128, 4096], BF16, tag=f"xt{scc}", name=f"xt{scc}")
                    for scc in range(SC)]
            qidx_t = consts.tile([128, SC * (cap // 16)], I16, tag="qi",
                                 name="qi")

            # all bulk input goes on the sync (SP) hardware-dynamic queue:
            # it fans out across 6+ DMA engines at ~150 GB/s, unlike the
            # scalar/gpsimd queues which drain through a single engine
            nc.sync.dma_start(out=wq_t[0], in_=wq[0:128, :])
            nc.sync.dma_start(out=xt_t[0], in_=xt[0:128, :])
            nc.sync.dma_start(out=wk_t[0], in_=wk[0:128, :])
            nc.sync.dma_start(out=xt_t[1], in_=xt[128:256, :])
            nc.sync.dma_start(out=wq_t[1], in_=wq[128:256, :])
            nc.sync.dma_start(out=wk_t[1], in_=wk[128:256, :])
            nc.sync.dma_start(out=xt_t[2], in_=xt[256:384, :])
            nc.sync.dma_start(out=xt_t[3], in_=xt[384:512, :])

            bq_t, bk_t = [], []
            for g in range(NG):
                bqt = consts.tile([128, 1], F32, tag=f"bq{g}", name=f"bq{g}")
                nc.gpsimd.dma_start(
                    out=bqt,
                    in_=bq[g * 128:(g + 1) * 128].rearrange("(p o) -> p o", o=1))
                bq_t.append(bqt)
                bkt = consts.tile([128, 1], F32, tag=f"bk{g}", name=f"bk{g}")
                nc.gpsimd.dma_start(
                    out=bkt,
                    in_=bk[g * 128:(g + 1) * 128].rearrange("(p o) -> p o", o=1))
                bk_t.append(bkt)
            nc.gpsimd.dma_start(out=qidx_t, in_=qidx[:, :])

            # identity for the small V transposes
            from concourse.masks import make_identity
            ident = consts.tile([128, 128], F32)
            make_identity(nc, ident)

            # ---- persistent activations ----
            tqf = [big.tile([128, S], F32, tag=f"tqf{g}", name=f"tqf{g}")
                   for g in range(NG)]
            qg = [big.tile([128, NQ], F32, tag=f"qg{g}", name=f"qg{g}")
                  for g in range(NG)]
            # qtp: packed q per head, bf16; the other head's rows stay zero
            # so the scores matmuls run a full K=128 contraction
            qtp = [[big.tile([128, NQ], BF16, tag=f"qtp{g}_{hh}",
                             name=f"qtp{g}_{hh}") for hh in range(2)]
                   for g in range(NG)]
            for g in range(NG):
                nc.vector.memset(qtp[g][0][64:128, :], 0.0)
                nc.vector.memset(qtp[g][1][0:64, :], 0.0)
            kt = [[big.tile([128, 512], BF16, tag=f"kt{g}_{sc}",
                            name=f"kt{g}_{sc}") for sc in range(SC)]
                  for g in range(NG)]
            # vp: v^T per head per key chunk + a ones column (denominator)
            vp = [[big.tile([128, 65], BF16, tag=f"vp{h}_{kc}",
                            name=f"vp{h}_{kc}") for kc in range(KC)]
                  for h in range(HPC)]

            vts_hist = {}

            def proj_group(g):
                """Projections for group g.  DVE order is arranged so no
                DVE instruction ever waits cross-engine: vts lands early
                (the PE transposes wait on its completion count) and the
                qtp casts trail their gathers by one chunk."""
                for sc in range(SC):
                    ssl = slice(sc * 512, (sc + 1) * 512)
                    csl = slice(sc * cap, (sc + 1) * cap)
                    isl = slice(sc * (cap // 16), (sc + 1) * (cap // 16))
                    pq = ps_m.tile([128, 512], F32, tag="ep", name=f"pq{g}_{sc}")
                    for dc in range(8):
                        nc.tensor.matmul(pq[:, 0:512],
                                         wq_t[g][:, dc * 128:(dc + 1) * 128],
                                         xt_t[sc][:, dc * 512:(dc + 1) * 512],
                                         start=(dc == 0), stop=(dc == 7))
                    pk = ps_m.tile([128, 512], F32, tag="ep", name=f"pk{g}_{sc}")
                    for dc in range(8):
                        nc.tensor.matmul(pk[:, 0:512],
                                         wk_t[g][:, dc * 128:(dc + 1) * 128],
                                         xt_t[sc][:, dc * 512:(dc + 1) * 512],
                                         start=(dc == 0), stop=(dc == 7))
                    nc.vector.tensor_scalar_add(tqf[g][:, ssl], pq[:, 0:512],
                                                bq_t[g])
                    tk = tmp.tile([128, 512], F32, tag="tk", name=f"tk{g}_{sc}")
                    nc.vector.tensor_scalar_add(tk, pk[:, 0:512], bk_t[g])
                    # v = q + k (early in DVE order: vtrans waits on it)
                    vts = tmp.tile([128, 512], F32, tag="vts",
                                   name=f"vts{g}_{sc}")
                    nc.vector.tensor_add(vts, tqf[g][:, ssl], tk)
                    vts_hist[(g, sc)] = vts
                    # kt = softplus(k) = ln(exp(k) + 1), one resident table
                    te = tmp.tile([128, 512], F32, tag="te", name=f"te{g}_{sc}")
                    nc.scalar.activation(out=te, in_=tk, func=AF.Exp)
                    nc.scalar.activation(out=kt[g][sc], in_=te,
                                         func=AF.Ln, bias=1.0)
                    # packed-q gather (gpsimd)
                    nc.gpsimd.ap_gather(
                        qg[g][:, csl], tqf[g][:, ssl], qidx_t[:, isl],
                        channels=128, num_elems=512, d=1, num_idxs=cap)
                    if sc > 0:
                        emit_vtrans(g, sc - 1)
                        emit_qtp(g, sc - 1)
                    if sc == SC - 1:
                        emit_vtrans(g, sc)
                        emit_qtp(g, sc)
                if zero_col0:
                    nc.vector.memset(qtp[g][0][0:64, 0:1], 0.0)
                    nc.vector.memset(qtp[g][1][64:128, 0:1], 0.0)

            def emit_vtrans(g, sc):
                for jj in range(4):
                    j = sc * 4 + jj
                    pv = ps_m.tile([128, 128], F32, tag="ep",
                                   name=f"pv{g}_{j}")
                    nc.tensor.transpose(pv,
                                        vts_hist[(g, sc)][:, jj * 128:(jj + 1) * 128],
                                        ident)
                    for hh in range(2):
                        h = g * 2 + hh
                        nc.vector.tensor_copy(vp[h][j][:, 0:64],
                                              pv[:, hh * 64:(hh + 1) * 64])
                        nc.vector.memset(vp[h][j][:, 64:65], 1.0)

            def emit_qtp(g, sc):
                csl = slice(sc * cap, (sc + 1) * cap)
                nc.vector.tensor_copy(qtp[g][0][0:64, csl], qg[g][0:64, csl])
                nc.vector.tensor_copy(qtp[g][1][64:128, csl],
                                      qg[g][64:128, csl])

            def make_epilogue(g, qc, csA, csB):
                # everything here reads the SBUF copies, so it can run long
                # after the PSUM accumulators were reused
                def emit(chunks):
                    # 1/den = exp(-ln(den)) on the scalar engine (resident
                    # table, slots into the exp stream), broadcast on
                    # gpsimd, multiply on DVE (idle during serial attn),
                    # store via the sync queue.
                    for c0, c1 in chunks:
                        w = c1 - c0
                        bcs = []
                        for hh, cs in ((0, csA), (1, csB)):
                            lnd = ep.tile([1, 512], F32, tag=f"l{hh}",
                                          name=f"l{g}_{qc}_{hh}_{c0}")
                            nc.scalar.activation(out=lnd[:, 0:w],
                                                 in_=cs[64:65, c0:c1],
                                                 func=AF.Ln)
                            rcp = ep.tile([1, 512], F32, tag=f"r{hh}",
                                          name=f"r{g}_{qc}_{hh}_{c0}")
                            nc.scalar.activation(out=rcp[:, 0:w],
                                                 in_=lnd[:, 0:w],
                                                 func=AF.Exp, scale=-1.0)
                            bc = ep.tile([64, 512], F32, tag=f"b{hh}",
                                         name=f"b{g}_{qc}_{hh}_{c0}")
                            nc.gpsimd.partition_broadcast(bc[:, 0:w],
                                                          rcp[0:1, 0:w])
                            bcs.append(bc)
                        for hh, cs in ((0, csA), (1, csB)):
                            cf = ep.tile([64, 512], F32, tag=f"cf{hh}",
                                         name=f"cf{g}_{qc}_{hh}_{c0}")
                            nc.vector.tensor_mul(cf[:, 0:w], cs[0:64, c0:c1],
                                                 bcs[hh][:, 0:w])
                            row = (g * 2 + hh) * 64
                            nc.sync.dma_start(
                                out=out[row:row + 64,
                                        qc * 512 + c0:qc * 512 + c1],
                                in_=cf[:, 0:w])
                return emit

            def attn_group(g, pending):
                """Attention for group g (scalar-paced steady state)."""
                vpA = vp[g * 2]
                vpB = vp[g * 2 + 1]
                ktg = kt[g]
                for qc in range(SQC):
                    qsl = slice(qc * 512, (qc + 1) * 512)
                    cA = ps_c.tile([65, 512], F32, tag="cA", name=f"cA{g}_{qc}")
                    cB = ps_c.tile([65, 512], F32, tag="cB", name=f"cB{g}_{qc}")
                    prev = None
                    for kc in range(KC):
                        sAB = ps_s.tile([128, 1024], F32, tag="sAB",
                                        name=f"s{g}_{qc}_{kc}")
                        lhs = ktg[kc // 4][:, (kc % 4) * 128:(kc % 4 + 1) * 128]
                        nc.tensor.matmul(sAB[:, 0:512], lhs,
                                         qtp[g][0][:, qsl],
                                         start=True, stop=True)
                        nc.tensor.matmul(sAB[:, 512:1024], lhs,
                                         qtp[g][1][:, qsl],
                                         start=True, stop=True)
                        eAB = expp.tile([128, 1024], BF16, tag="eAB",
                                        name=f"e{g}_{qc}_{kc}")
                        nc.scalar.activation(out=eAB, in_=sAB, func=AF.Exp,
                                             scale=0.125)
                        if kc == 8 and pending[0] is not None:
                            # previous qc's deferred normalize
                            pending[0]([(0, 512)])
                            pending[0] = None
                        # ctx runs one kc behind so the PE never waits on exp
                        if prev is not None:
                            pe, pkc = prev
                            nc.tensor.matmul(cA, vpA[pkc], pe[:, 0:512],
                                             start=(pkc == 0), stop=False)
                            nc.tensor.matmul(cB, vpB[pkc], pe[:, 512:1024],
                                             start=(pkc == 0), stop=False)
                        prev = (eAB, kc)
                    pe, pkc = prev
                    nc.tensor.matmul(cA, vpA[pkc], pe[:, 0:512],
                                     start=False, stop=True)
                    nc.tensor.matmul(cB, vpB[pkc], pe[:, 512:1024],
                                     start=False, stop=True)
                    # copy ctx+denominator to SBUF immediately: this frees
                    # the PSUM accumulator so the next qc never stalls; the
                    # slow normalize is deferred into the next qc
                    csA = ep.tile([65, 512], F32, tag="cs0",
                                  name=f"cs{g}_{qc}_0")
                    nc.vector.tensor_copy(csA, cA)
                    csB = ep.tile([65, 512], F32, tag="cs1",
                                  name=f"cs{g}_{qc}_1")
                    nc.vector.tensor_copy(csB, cB)
                    pending[0] = make_epilogue(g, qc, csA, csB)

            pending = [None]
            for g in range(NG):
                proj_group(g)
                attn_group(g, pending)
            # tail: the last qc's normalize, chunked so the scalar recip,
            # broadcast, multiply and store pipeline instead of serializing
            pending[0]([(0, 256), (256, 512)])

    nc.finalize()
    return nc


def _get_nc(cap, zero_col0):
    key = (cap, zero_col0)
    if key not in _CACHE:
        _CACHE[key] = _build(cap, zero_col0)
    return _CACHE[key]


def _pack_queries(am_row, cap):
    """Build per-512-chunk packed index lists for one batch.

    Packed column c*cap+s <- query (c*512 + idx[c][s]).  Column 0 is
    always reserved and zeroed on the device; every masked query position
    takes its output from column 0 on the host (a zeroed q column yields
    the uniform-softmax result, identical for all masked queries).

    Returns (fits, wrapped_idx [128, SC*cap//16] int16, scatter info).
    """
    masked = np.where(am_row == 0)[0]
    cols = []          # packed column (valid entries, in order)
    qpos = []          # matching global query index
    wrapped = np.zeros((SC, 128, cap // 16), dtype=np.int16)
    for c in range(SC):
        lo = c * 512
        un = np.where(am_row[lo:lo + 512] == 1)[0]    # local indices
        reserve = 1 if c == 0 else 0
        if len(un) + reserve > cap:
            return False, None, None
        idx = np.zeros(cap, dtype=np.int16)           # pad/rep = 0 (valid)
        idx[reserve:reserve + len(un)] = un
        # wrapped layout: index j lives at [j % 16, j // 16], replicated
        # into each 16-partition block
        wrapped[c] = np.tile(idx.reshape(cap // 16, 16).T, (8, 1))
        cols.extend(c * cap + reserve + i for i in range(len(un)))
        qpos.extend(lo + int(u) for u in un)
    info = {
        "cols": np.asarray(cols, dtype=np.int64),
        "qpos": np.asarray(qpos, dtype=np.int64),
        "masked": masked,
    }
    return True, wrapped.transpose(1, 0, 2).reshape(128, SC * (cap // 16)), info


def _shard_inputs(hidden_states, attention_mask, Wq, bq, Wk, bk):
    hs = np.asarray(hidden_states, dtype=np.float32)
    am = np.asarray(attention_mask)
    Wq = np.asarray(Wq, dtype=np.float32)
    Wk = np.asarray(Wk, dtype=np.float32)
    bq = np.asarray(bq, dtype=np.float32)
    bk = np.asarray(bk, dtype=np.float32)
    BF = ml_dtypes.bfloat16

    # packed capacity: 384/chunk for the random ~50% mask; escalate if a
    # chunk has too many unmasked queries (640 always fits: 512+1 <= 640)
    packs = None
    for cap in (384, 512, 640):
        packs = []
        for b in range(B):
            ok, wrapped, info = _pack_queries(am[b], cap)
            if not ok:
                packs = None
                break
            packs.append((wrapped, info))
        if packs is not None:
            break
    assert packs is not None

    _META.clear()
    _META["cap"] = cap
    _META["zero_col0"] = True
    _META["packs"] = packs

    # X^T packed per seq chunk: [sc][p, dc*512+s] = X[b, sc*512+s, dc*128+p]
    xts = [np.ascontiguousarray(
        hs[b].T.astype(BF).reshape(8, 128, SC, 512).transpose(2, 1, 0, 3)
        .reshape(SC * 128, 8 * 512)) for b in range(B)]

    in_maps = []
    for c in range(NCORES):
        b = c // (NCORES // B)
        hg = c % (NCORES // B)
        cols = slice(hg * 2 * 128, (hg + 1) * 2 * 128)

        def _tile_w(W):
            # [g][p, dc*128+j] = W[dc*128+p, cols[g*128+j]]
            return np.ascontiguousarray(
                W[:, cols].astype(BF).reshape(8, 128, NG, 128)
                .transpose(2, 1, 0, 3).reshape(NG * 128, 8 * 128))
        in_maps.append({
            "xt": xts[b],
            "wq": _tile_w(Wq),
            "wk": _tile_w(Wk),
            "bq": np.ascontiguousarray(bq[cols]),
            "bk": np.ascontiguousarray(bk[cols]),
            "qidx": packs[b][0],
        })
    return in_maps


def _gather(results):
    cap = _META["cap"]
    NQ = cap * SC
    full = np.empty((B, S, D), dtype=np.float32)
    for c in range(NCORES):
        b = c // (NCORES // B)
        hg = c % (NCORES // B)
        _, info = _META["packs"][b]
        r = results[c]["out"].reshape(HPC, HD, NQ)
        for h in range(HPC):
            col = hg * 2 * 128 + h * 64
            blk = full[b, :, col:col + 64]
            blk[info["qpos"], :] = r[h][:, info["cols"]].T
            if len(info["masked"]):
                blk[info["masked"], :] = r[h][:, 0]
    return full


def run_sharded(in_maps, **kw):
    from concourse.bass_utils import run_bass_kernel_spmd
    nc = _get_nc(_META["cap"], _META["zero_col0"])
    return run_bass_kernel_spmd(nc, in_maps, list(range(NCORES)), **kw)


def kernel(hidden_states, attention_mask, Wq, bq, Wk, bk):
    in_maps = _shard_inputs(hidden_states, attention_mask, Wq, bq, Wk, bk)
    res = run_sharded(in_maps)
    return _gather(res.results)
